# Initial kernel scaffold
#
"""GQA kernel for Trainium2, 8 NeuronCores (raw bass, manual sync).

Core c handles batch b=c//4, head-group hg=c%4 (8 q-heads, 2 kv-heads).
All device math in transposed [feature, T] layouts (host pre-transposes x,
host post-transposes/sums the output):
  qT = Wq.T @ xT ; RoPE via pair-swap permutation matmuls + cos/sin DVE
  ST[k,q] = krot_chunk.T @ qrot  (== P.T, so AV needs no transpose)
  PT = exp(ST)  (no max subtraction — scores are provably small)
  OT[d,q] accum = Vaug.T @ PT  (ones column in Vaug -> softmax sums)
  outT = Wo.T @ OTstack ; host sums 4 partials per batch + transposes.

PSUM bank map (8 slots of [128,512]f32):
  phase A q: accum 0-3, swap 4-7
  phase A k: accum 0-3, rope ping-pong (4,5)/(6,7)
  phase A v: accum 0-3 (after k evac)
  phase B:   st pairs (0,1)/(2,3), OT accum 4/5
  phase C:   lift 6/7, oproj 0/1
"""
import numpy as np
import ml_dtypes

import concourse.bass as bass
import concourse.mybir as mybir
from concourse.bass_utils import run_bass_kernel_spmd

B, T, E = 2, 2048, 2048
H, KV, D = 32, 8, 64
HL, KVL = 8, 2
QC, EC, TJ, TC = 4, 16, 4, 16
BF = mybir.dt.bfloat16
F32 = mybir.dt.float32

_CACHE = {}


def _build():
    nc = bass.Bass("TRN2", target_bir_lowering=False, debug=False, num_devices=8)
    dr = {}
    dr["xt"] = nc.dram_tensor("xt", [E, T], BF, kind="ExternalInput")
    dr["wq"] = nc.dram_tensor("wq", [E, HL * D], BF, kind="ExternalInput")
    dr["wk"] = nc.dram_tensor("wk", [E, KVL * D], BF, kind="ExternalInput")
    dr["wv"] = nc.dram_tensor("wv", [E, KVL * D], BF, kind="ExternalInput")
    dr["wo"] = nc.dram_tensor("wo", [HL * D, E], BF, kind="ExternalInput")
    dr["cc"] = nc.dram_tensor("cc", [128, T], BF, kind="ExternalInput")
    dr["ss"] = nc.dram_tensor("ss", [128, T], BF, kind="ExternalInput")
    dr["perm"] = nc.dram_tensor("perm", [128, 5, 128], BF, kind="ExternalInput")
    dr["msk"] = nc.dram_tensor("msk", [128, 4, 512], BF, kind="ExternalInput")
    dr["lift"] = nc.dram_tensor("lift", [64, 2, 128], BF, kind="ExternalInput")
    dr["outT"] = nc.dram_tensor("outT", [E, T], F32, kind="ExternalOutput")

    from contextlib import ExitStack
    with ExitStack() as _ctx:
        xt_sb = _ctx.enter_context(nc.sbuf_tensor("xt_sb", [128, EC, T], BF))
        wq_sb = _ctx.enter_context(nc.sbuf_tensor("wq_sb", [128, EC, HL * D], BF))
        wk_sb = _ctx.enter_context(nc.sbuf_tensor("wk_sb", [128, EC, KVL * D], BF))
        wv_sb = _ctx.enter_context(nc.sbuf_tensor("wv_sb", [128, EC, KVL * D], BF))
        wo_sb = _ctx.enter_context(nc.sbuf_tensor("wo_sb", [128, QC, E], BF))
        cc_sb = _ctx.enter_context(nc.sbuf_tensor("cc_sb", [128, T], BF))
        ss_sb = _ctx.enter_context(nc.sbuf_tensor("ss_sb", [128, T], BF))
        perm_sb = _ctx.enter_context(nc.sbuf_tensor("perm_sb", [128, 5, 128], BF))
        msk_sb = _ctx.enter_context(nc.sbuf_tensor("msk_sb", [128, 4, 512], BF))
        lift_sb = _ctx.enter_context(nc.sbuf_tensor("lift_sb", [64, 2, 128], BF))
        qraw_sb = _ctx.enter_context(nc.sbuf_tensor("qraw_sb", [128, T], BF))
        kraw_sb = _ctx.enter_context(nc.sbuf_tensor("kraw_sb", [128, T], BF))
        t1_sb = _ctx.enter_context(nc.sbuf_tensor("t1_sb", [128, T], BF))
        t2_sb = _ctx.enter_context(nc.sbuf_tensor("t2_sb", [128, T], BF))
        qrot_sb = _ctx.enter_context(nc.sbuf_tensor("qrot_sb", [128, QC, T], BF))
        krot_sb = _ctx.enter_context(nc.sbuf_tensor("krot_sb", [128, KVL, T], BF))
        vaug_sb = _ctx.enter_context(nc.sbuf_tensor("vaug_sb", [128, TC, KVL, 65], BF))
        pt_sb = _ctx.enter_context(nc.sbuf_tensor("pt_sb", [128, 2, 1024], BF))
        rs_sb = _ctx.enter_context(nc.sbuf_tensor("rs_sb", [128, 512], F32))
        rrep_sb = _ctx.enter_context(nc.sbuf_tensor("rrep_sb", [64, 2, 512], BF))
        rbf_sb = _ctx.enter_context(nc.sbuf_tensor("rbf_sb", [128, 512], BF))
        ones_sb = _ctx.enter_context(nc.sbuf_tensor("ones_sb", [128, 64], BF))
        ot64_sb = _ctx.enter_context(nc.sbuf_tensor("ot64_sb", [64, QC, T], BF))
        ot128_sb = _ctx.enter_context(nc.sbuf_tensor("ot128_sb", [128, QC, T], BF))
        ostg_sb = _ctx.enter_context(nc.sbuf_tensor("ostg_sb", [128, 2, 512], F32))
        ps = _ctx.enter_context(nc.psum_tensor("ps", [128, 8, 512], F32))
        dma_sem = _ctx.enter_context(nc.semaphore("dma_sem"))
        pe_sem = _ctx.enter_context(nc.semaphore("pe_sem"))
        v_sem = _ctx.enter_context(nc.semaphore("v_sem"))
        s_sem = _ctx.enter_context(nc.semaphore("s_sem"))
        block = _ctx.enter_context(nc.Block())
        sems = {"pe": pe_sem, "v": v_sem, "s": s_sem, "dma": dma_sem}
        ops = []

        def op(eng, fn, deps=(), name=None, inc=0):
            ops.append((eng, fn, inc, tuple(deps), name))

        MM = nc.tensor.matmul

        def sl(i):
            return ps[:, i, :]

        def ps4(lo):   # 4 contiguous slots as [128, 2048]
            return ps[:, lo:lo + 4, :].rearrange("p a b -> p (a b)")

        def ps2(lo):   # 2 contiguous slots as [128, 1024]
            return ps[:, lo:lo + 2, :].rearrange("p a b -> p (a b)")

        # ======== DMA loads ========
        loads = [
            (xt_sb[:, :, :], dr["xt"].rearrange("(ec p) t -> p ec t", p=128)),
            (wq_sb[:, :, :], dr["wq"].rearrange("(ec p) m -> p ec m", p=128)),
            (wk_sb[:, :, :], dr["wk"].rearrange("(ec p) m -> p ec m", p=128)),
            (wv_sb[:, :, :], dr["wv"].rearrange("(ec p) m -> p ec m", p=128)),
            (wo_sb[:, :, :], dr["wo"].rearrange("(qc p) e -> p qc e", p=128)),
            (cc_sb[:, :], dr["cc"][:, :]),
            (ss_sb[:, :], dr["ss"][:, :]),
            (perm_sb[:, :, :], dr["perm"][:, :, :]),
            (msk_sb[:, :, :], dr["msk"][:, :, :]),
            (lift_sb[:, :, :], dr["lift"][:, :, :]),
        ]
        for li, (o, i) in enumerate(loads):
            op("dma", (lambda o=o, i=i: nc.sync.dma_start(out=o, in_=i)),
               name=f"load_{li}", inc=16)
        LD = [f"load_{li}" for li in range(len(loads))]

        op("v", lambda: nc.vector.memset(vaug_sb[:, :, :, 64:65], 1.0),
           name="vones", inc=1)
        op("v", lambda: nc.vector.memset(ones_sb[:, :], 1.0),
           name="vones2", inc=1)

        # ======== phase A: q projections + rope ========
        for c in range(QC):
            deps = [LD[0], LD[1]]
            if c > 0:
                deps += [f"qm1_{c-1}", f"qm2_{c-1}"]
            for tj in range(TJ):
                for e in range(EC):
                    op("pe", (lambda c=c, tj=tj, e=e: MM(
                        out=sl(tj),
                        lhsT=wq_sb[:, e, c * 128:(c + 1) * 128],
                        rhs=xt_sb[:, e, tj * 512:(tj + 1) * 512],
                        start=(e == 0), stop=(e == EC - 1))),
                       deps=(deps if (tj == 0 and e == 0) else ()),
                       name=(f"qps_{c}_{tj}" if e == EC - 1 else None),
                       inc=(1 if e == EC - 1 else 0))
            for tj in range(TJ):
                edeps = [f"qps_{c}_{tj}"]
                if tj == 0 and c > 0:
                    edeps.append(f"qsw_{c-1}")
                op("v", (lambda tj=tj: nc.vector.tensor_copy(
                    qraw_sb[:, tj * 512:(tj + 1) * 512], sl(tj))),
                   deps=edeps,
                   name=(f"qraw_{c}" if tj == TJ - 1 else None),
                   inc=(1 if tj == TJ - 1 else 0))
            sdeps = [f"qraw_{c}", LD[7]] + ([f"qm2_{c-1}"] if c > 0 else [])
            for tj in range(TJ):
                op("pe", (lambda tj=tj: MM(
                    out=sl(4 + tj),
                    lhsT=perm_sb[:, 0, :],
                    rhs=qraw_sb[:, tj * 512:(tj + 1) * 512],
                    start=True, stop=True)),
                   deps=(sdeps if tj == 0 else ()),
                   name=(f"qsw_{c}" if tj == TJ - 1 else None),
                   inc=(1 if tj == TJ - 1 else 0))
            op("v", (lambda: nc.vector.tensor_mul(
                t1_sb[:, :], ps4(0), cc_sb[:, :])),
               deps=[LD[5]], name=f"qm1_{c}", inc=1)
            op("v", (lambda: nc.vector.tensor_mul(
                t2_sb[:, :], ps4(4), ss_sb[:, :])),
               deps=[f"qsw_{c}", LD[6]], name=f"qm2_{c}", inc=1)
            op("v", (lambda c=c: nc.vector.tensor_add(
                qrot_sb[:, c, :], t1_sb[:, :], t2_sb[:, :])),
               name=f"qrot_{c}", inc=1)

        # ======== phase A: k projections + dup/rope per (g, tj) ========
        kdeps = [LD[2], f"qm1_{QC-1}"]
        for tj in range(TJ):
            for e in range(EC):
                op("pe", (lambda tj=tj, e=e: MM(
                    out=sl(tj),
                    lhsT=wk_sb[:, e, :],
                    rhs=xt_sb[:, e, tj * 512:(tj + 1) * 512],
                    start=(e == 0), stop=(e == EC - 1))),
                   deps=(kdeps if (tj == 0 and e == 0) else ()),
                   name=(f"kps_{tj}" if e == EC - 1 else None),
                   inc=(1 if e == EC - 1 else 0))
        for tj in range(TJ):
            op("v", (lambda tj=tj: nc.vector.tensor_copy(
                kraw_sb[:, tj * 512:(tj + 1) * 512], sl(tj))),
               deps=[f"kps_{tj}"],
               name=("kraw" if tj == TJ - 1 else None),
               inc=(1 if tj == TJ - 1 else 0))
        for g in range(KVL):
            for tj in range(TJ):
                i = g * TJ + tj
                pp = 4 + 2 * (i % 2)
                ddeps = ["kraw"]
                if i == 0:
                    ddeps.append(f"qm2_{QC-1}")
                elif i == 1:
                    pass
                if i >= 2:
                    ddeps.append(f"kro_{(i-2)//TJ}_{(i-2)%TJ}")
                op("pe", (lambda g=g, tj=tj, pp=pp: MM(
                    out=sl(pp),
                    lhsT=perm_sb[:, 1 + g * 2, :],
                    rhs=kraw_sb[:, tj * 512:(tj + 1) * 512],
                    start=True, stop=True)),
                   deps=ddeps, inc=0)
                op("pe", (lambda g=g, tj=tj, pp=pp: MM(
                    out=sl(pp + 1),
                    lhsT=perm_sb[:, 2 + g * 2, :],
                    rhs=kraw_sb[:, tj * 512:(tj + 1) * 512],
                    start=True, stop=True)),
                   name=f"kdup_{g}_{tj}", inc=1)
                op("v", (lambda pp=pp, tj=tj: nc.vector.tensor_mul(
                    t1_sb[:, 0:512], sl(pp),
                    cc_sb[:, tj * 512:(tj + 1) * 512])),
                   deps=[f"kdup_{g}_{tj}"], inc=0)
                op("v", (lambda pp=pp, tj=tj: nc.vector.tensor_mul(
                    t2_sb[:, 0:512], sl(pp + 1),
                    ss_sb[:, tj * 512:(tj + 1) * 512])),
                   inc=0)
                op("v", (lambda g=g, tj=tj: nc.vector.tensor_add(
                    krot_sb[:, g, tj * 512:(tj + 1) * 512],
                    t1_sb[:, 0:512], t2_sb[:, 0:512])),
                   name=f"kro_{g}_{tj}", inc=1)

        # ======== phase A: v projections (slots 0-3 after k evac) ========
        for tt in range(4):
            vdeps = [LD[3], "vones", "kraw"]
            if tt > 0:
                vdeps.append(f"vevac_{tt-1}")
            for t4 in range(4):
                t = tt * 4 + t4
                for e in range(EC):
                    op("pe", (lambda t=t, t4=t4, e=e: MM(
                        out=ps[:, t4, 0:128],
                        lhsT=xt_sb[:, e, t * 128:(t + 1) * 128],
                        rhs=wv_sb[:, e, :],
                        start=(e == 0), stop=(e == EC - 1))),
                       deps=(vdeps if (t4 == 0 and e == 0) else ()),
                       name=(f"vps_{tt}" if (t4 == 3 and e == EC - 1) else None),
                       inc=(1 if (t4 == 3 and e == EC - 1) else 0))
            for t4 in range(4):
                t = tt * 4 + t4
                for g in range(KVL):
                    op("v", (lambda t=t, t4=t4, g=g: nc.vector.tensor_copy(
                        vaug_sb[:, t, g, 0:64],
                        ps[:, t4, g * 64:(g + 1) * 64])),
                       deps=([f"vps_{tt}"] if (t4 == 0 and g == 0) else ()),
                       name=(f"vevac_{tt}" if (t4 == 3 and g == 1) else None),
                       inc=(1 if (t4 == 3 and g == 1) else 0))

        # ======== phase B: attention ========
        last_exp = None
        for lh in range(HL):
            g, pq, c = lh // 4, lh % 2, lh // 2
            for qj in range(TJ):
                idx = lh * TJ + qj
                nk = 4 * qj + 4
                npair = nk // 2
                oslot = 4 + idx % 2
                pidx = idx - 1
                pnp = 2 * ((pidx % TJ) + 1) if pidx >= 0 else 0
                plh, pqj = pidx // TJ, pidx % TJ
                for kp in range(npair):
                    sbase = 2 * (kp % 2)
                    stdeps = []
                    if idx == 0 and kp == 0:
                        stdeps += [f"vevac_3", f"kro_{KVL-1}_{TJ-1}",
                                   f"qrot_{QC-1}"]
                    if idx > 0 and kp == 0:
                        stdeps.append(f"exp_{plh}_{pqj}_{pnp-1}")
                    if kp >= 2:
                        stdeps.append(f"exp_{lh}_{qj}_{kp-2}")
                    for j in range(2):
                        ki = kp * 2 + j
                        op("pe", (lambda sbase=sbase, j=j, ki=ki, pq=pq, g=g, c=c, qj=qj: MM(
                            out=sl(sbase + j),
                            lhsT=krot_sb[pq * 64:(pq + 1) * 64, g,
                                         ki * 128:(ki + 1) * 128],
                            rhs=qrot_sb[pq * 64:(pq + 1) * 64, c,
                                        qj * 512:(qj + 1) * 512],
                            start=True, stop=True)),
                           deps=(stdeps if j == 0 else ()),
                           name=(f"st_{lh}_{qj}_{kp}" if j == 1 else None),
                           inc=(1 if j == 1 else 0))
                    edeps = [f"st_{lh}_{qj}_{kp}"]
                    if kp >= 2:
                        edeps.append(f"av_{lh}_{qj}_{kp-2}")
                    if kp < 2 and idx > 0:
                        edeps.append(f"av_{plh}_{pqj}_{pnp-1}")
                    op("s", (lambda sbase=sbase, kp=kp: nc.scalar.activation(
                        out=pt_sb[:, kp % 2, :],
                        in_=ps2(sbase),
                        func=mybir.ActivationFunctionType.Exp,
                        scale=1.0)),
                       deps=edeps, name=f"exp_{lh}_{qj}_{kp}", inc=1)
                    last_exp = f"exp_{lh}_{qj}_{kp}"
                    avdep = f"exp_{lh}_{qj}_{kp}"
                    if kp * 2 + 1 >= 4 * qj:
                        first = True
                        for j in range(2):
                            ki = kp * 2 + j
                            r = ki - 4 * qj
                            if r >= 0:
                                op("v", (lambda kp=kp, j=j, r=r: nc.vector.tensor_mul(
                                    pt_sb[:, kp % 2, j * 512:(j + 1) * 512],
                                    pt_sb[:, kp % 2, j * 512:(j + 1) * 512],
                                    msk_sb[:, r, :])),
                                   deps=([avdep, LD[8]] if first else ()),
                                   name=(f"msk_{lh}_{qj}_{kp}" if j == 1 else None),
                                   inc=(1 if j == 1 else 0))
                                first = False
                        avdep = f"msk_{lh}_{qj}_{kp}"
                    avdeps = [avdep]
                    if kp == 0 and idx >= 2:
                        avdeps.append(f"onorm_{idx-2}")
                    for j in range(2):
                        ki = kp * 2 + j
                        op("pe", (lambda oslot=oslot, kp=kp, j=j, ki=ki, g=g, nk=nk: MM(
                            out=ps[0:65, oslot, :],
                            lhsT=vaug_sb[:, ki, g, :],
                            rhs=pt_sb[:, kp % 2, j * 512:(j + 1) * 512],
                            start=(ki == 0), stop=(ki == nk - 1))),
                           deps=(avdeps if j == 0 else ()),
                           name=(f"av_{lh}_{qj}_{kp}" if j == 1 else None),
                           inc=(1 if j == 1 else 0))
                rdeps = [f"av_{lh}_{qj}_{npair-1}"]
                if idx > 0:
                    rdeps.append(f"rbc_{idx-1}")
                op("v", (lambda oslot=oslot: nc.vector.reciprocal(
                    out=rs_sb[64:65, :], in_=ps[64:65, oslot, :])),
                   deps=rdeps, inc=0)
                op("v", (lambda: nc.vector.tensor_copy(
                    rbf_sb[64:65, :], rs_sb[64:65, :])),
                   name=f"rcp_{idx}", inc=1)
                bdeps = [f"rcp_{idx}", "vones2"]
                if idx < 2:
                    bdeps.append(f"kro_{KVL-1}_{TJ-1}")
                if idx >= 2:
                    bdeps.append(f"onorm_{idx-2}")
                bslot = 6 + idx % 2
                op("pe", (lambda bslot=bslot: MM(
                    out=ps[0:64, bslot, :],
                    lhsT=ones_sb[64:65, 0:64],
                    rhs=rbf_sb[64:65, :],
                    start=True, stop=True)),
                   deps=bdeps, name=f"rbc_{idx}", inc=1)
                op("v", (lambda idx=idx, bslot=bslot: nc.vector.tensor_copy(
                    rrep_sb[:, idx % 2, :], ps[0:64, bslot, :])),
                   deps=[f"rbc_{idx}"], inc=0)
                if lh % 2 == 0:
                    tgt = ot128_sb[0:64, lh // 2, qj * 512:(qj + 1) * 512]
                else:
                    tgt = ot64_sb[:, lh // 2, qj * 512:(qj + 1) * 512]
                op("v", (lambda tgt=tgt, oslot=oslot, idx=idx: nc.vector.tensor_mul(
                    tgt,
                    ps[0:64, oslot, :],
                    rrep_sb[:, idx % 2, :])),
                   deps=[f"rbc_{idx}"], name=f"onorm_{idx}", inc=1)

        # ======== phase C: lift + output projection ========
        for cq in range(QC):
            for tj in range(TJ):
                idx = cq * TJ + tj
                ldeps = [f"onorm_{(2*cq+1)*TJ+tj}", LD[9],
                         f"onorm_{HL*TJ-1}"]
                if idx >= 2:
                    ldeps.append(f"lifte_{idx-2}")
                op("pe", (lambda cq=cq, tj=tj, idx=idx: MM(
                    out=sl(6 + idx % 2),
                    lhsT=lift_sb[:, 1, :],
                    rhs=ot64_sb[:, cq, tj * 512:(tj + 1) * 512],
                    start=True, stop=True)),
                   deps=ldeps, name=f"lift_{cq}_{tj}", inc=1)
                op("v", (lambda cq=cq, tj=tj, idx=idx: nc.vector.tensor_copy(
                    ot128_sb[64:128, cq, tj * 512:(tj + 1) * 512],
                    ps[64:128, 6 + idx % 2, :])),
                   deps=[f"lift_{cq}_{tj}"], name=f"lifte_{idx}", inc=1)

        for m in range(EC):
            for tj in range(TJ):
                idx = m * TJ + tj
                odeps = [f"lifte_{cq * TJ + tj}" for cq in range(QC)] + [LD[4]]
                odeps += [f"onorm_{(2 * cq) * TJ + tj}" for cq in range(QC)]
                if idx < 2:
                    odeps.append(last_exp)
                if idx >= 2:
                    odeps.append(f"oevac_{idx-2}")
                for kc in range(QC):
                    op("pe", (lambda m=m, tj=tj, kc=kc, idx=idx: MM(
                        out=sl(idx % 2),
                        lhsT=wo_sb[:, kc, m * 128:(m + 1) * 128],
                        rhs=ot128_sb[:, kc, tj * 512:(tj + 1) * 512],
                        start=(kc == 0), stop=(kc == QC - 1))),
                       deps=(odeps if kc == 0 else ()),
                       name=(f"oproj_{m}_{tj}" if kc == QC - 1 else None),
                       inc=(1 if kc == QC - 1 else 0))
                edeps = [f"oproj_{m}_{tj}"]
                if idx >= 2:
                    edeps.append(f"ostore_{idx-2}")
                op("v", (lambda idx=idx: nc.vector.tensor_copy(
                    ostg_sb[:, idx % 2, :], sl(idx % 2))),
                   deps=edeps, name=f"oevac_{idx}", inc=1)
                op("dma", (lambda m=m, tj=tj, idx=idx: nc.sync.dma_start(
                    out=dr["outT"][m * 128:(m + 1) * 128,
                                   tj * 512:(tj + 1) * 512],
                    in_=ostg_sb[:, idx % 2, :])),
                   deps=[f"oevac_{idx}"], name=f"ostore_{idx}", inc=16)

        # ---------- resolve ticks ----------
        tick = {"pe": 0, "v": 0, "s": 0, "dma": 0}
        ev = {}
        for eng, fn, inc, deps, name in ops:
            tick[eng] += inc
            if name:
                ev[name] = (eng, tick[eng])
        total = dict(tick)

        def emit(block_eng, eng_key):
            waited = {}
            for eng, fn, inc, deps, name in ops:
                if eng != eng_key:
                    continue
                for d in deps:
                    deng, dtick = ev[d]
                    if deng == eng_key or waited.get(deng, -1) >= dtick:
                        continue
                    block_eng.wait_ge(sems[deng], dtick)
                    waited[deng] = dtick
                inst = fn()
                if inc:
                    inst.then_inc(sems[eng], inc)

        @block.sync
        def _(sync):
            emit(sync, "dma")
            sync.wait_ge(dma_sem, total["dma"])

        @block.tensor
        def _(tensor):
            emit(tensor, "pe")

        @block.vector
        def _(vector):
            emit(vector, "v")

        @block.scalar
        def _(scalar):
            emit(scalar, "s")

    return nc


def _host_tables():
    inv = 1.0 / (10000.0 ** (np.arange(0, D, 2, dtype=np.float64) / D))
    ang = np.arange(T, dtype=np.float64)[:, None] * inv[None, :]
    cos = np.cos(ang).T
    sin = np.sin(ang).T
    c64 = np.repeat(cos, 2, axis=0)
    s64 = np.empty((64, T))
    s64[0::2] = -sin
    s64[1::2] = sin
    cc = np.tile(c64, (2, 1)).astype(np.float32)
    ss = np.tile(s64, (2, 1)).astype(np.float32)

    def swap(r):
        return r + 1 if r % 2 == 0 else r - 1

    perm = np.zeros((128, 5, 128), np.float32)
    for r in range(128):
        perm[swap(r), 0, r] = 1.0
    for gg in range(2):
        for sw in range(2):
            for r in range(128):
                d_ = r % 64
                k = gg * 64 + (swap(d_) if sw else d_)
                perm[k, 1 + gg * 2 + sw, r] = 1.0

    msk = np.zeros((4, 128, 512), np.float32)
    for r in range(4):
        for p in range(128):
            v = r * 128 + p
            if v < 512:
                msk[r, p, v:] = 1.0
    msk = np.ascontiguousarray(msk.transpose(1, 0, 2))

    lift = np.zeros((64, 2, 128), np.float32)
    for d_ in range(64):
        lift[d_, 0, d_] = 1.0
        lift[d_, 1, 64 + d_] = 1.0
    return cc, ss, perm, msk, lift


def kernel(x, freq_cis, Wq, Wk, Wv, Wo):
    x = np.asarray(x, np.float32)
    Wq = np.asarray(Wq, np.float32)
    Wk = np.asarray(Wk, np.float32)
    Wv = np.asarray(Wv, np.float32)
    Wo = np.asarray(Wo, np.float32)
    bf16 = ml_dtypes.bfloat16

    if "nc" not in _CACHE:
        _CACHE["nc"] = _build()
    nc = _CACHE["nc"]

    cc, ss, perm, msk, lift = _host_tables()
    in_maps = []
    for core in range(8):
        b, hg = core // 4, core % 4
        in_maps.append({
            "xt": np.ascontiguousarray(x[b].T).astype(bf16),
            "wq": (Wq[:, hg * 512:(hg + 1) * 512] / 8.0).astype(bf16),
            "wk": Wk[:, hg * 128:(hg + 1) * 128].astype(bf16),
            "wv": Wv[:, hg * 128:(hg + 1) * 128].astype(bf16),
            "wo": Wo[hg * 512:(hg + 1) * 512, :].astype(bf16),
            "cc": cc.astype(bf16), "ss": ss.astype(bf16),
            "perm": perm.astype(bf16), "msk": msk.astype(bf16),
            "lift": lift.astype(bf16),
        })

    _CACHE["in_maps"] = in_maps
    res = run_bass_kernel_spmd(nc, in_maps, list(range(8)))
    out = np.zeros((B, T, E), np.float32)
    for core in range(8):
        b = core // 4
        out[b] += res.results[core]["outT"].T
    return out



# revision 10
# speedup vs baseline: 1.8184x; 1.8184x over previous
"""GQA kernel for Trainium2, 8 NeuronCores (raw bass, manual sync).

Core c handles batch b=c//4, head-group hg=c%4 (8 q-heads, 2 kv-heads).
All device math in transposed [feature, T] layouts (host pre-transposes x,
host post-transposes/sums the output):
  qT = Wq.T @ xT ; RoPE via pair-swap permutation matmuls + cos/sin DVE
  ST[k,q] = krot_chunk.T @ qrot  (== P.T, so AV needs no transpose)
  PT = exp(ST)  (no max subtraction -- scores are provably small)
  OT[d,q] accum = Vaug.T @ PT  (ones column in Vaug -> softmax sums)
  outT = Wo.T @ OTstack ; host sums 4 partials per batch + transposes.

Schedule (v2): software-pipelined so the PE never idles long enough for
the HAM clock gate to re-throttle:
  - input loads chunked per E-slice; Q-proj MMs dep on per-chunk sems
  - phase A: per-512-col PSUM ping-pong; evac/rope ops at [128,512] grain
  - phase B: flat chunk-pair pipeline; PE order [.., AV_p, ST_{p+2}, ..]
    so the exp (scalar ACT) of pair p overlaps AV_{p-1}+ST_{p+1} on PE
  - lift of odd-head outputs via SBUF->SBUF DMA (no PE matmuls)
  - phase C: oproj 4-bank rotation, 2-slot store staging

PSUM bank map (8 slots of [128,512]f32):
  A: qproj c -> bank tj, qswap -> 4+tj; kproj -> tj, kdup pairs (4,5)/(6,7);
     vproj -> bank t%4
  B: ST pair p -> banks (0,1)/(2,3) by p parity; OT accum 4/5 by idx parity;
     recip bcast 6/7 by idx parity
  C: oproj -> bank idx%4
"""
import numpy as np
import ml_dtypes

import concourse.bass as bass
import concourse.mybir as mybir
from concourse.bass_utils import run_bass_kernel_spmd

B, T, E = 2, 2048, 2048
H, KV, D = 32, 8, 64
HL, KVL = 8, 2
QC, EC, TJ, TC = 4, 16, 4, 16
BF = mybir.dt.bfloat16
F32 = mybir.dt.float32

_CACHE = {}


def _build():
    nc = bass.Bass("TRN2", target_bir_lowering=False, debug=False, num_devices=8)
    dr = {}
    dr["xt"] = nc.dram_tensor("xt", [E, T], BF, kind="ExternalInput")
    dr["wq"] = nc.dram_tensor("wq", [E, HL * D], BF, kind="ExternalInput")
    dr["wk"] = nc.dram_tensor("wk", [E, KVL * D], BF, kind="ExternalInput")
    dr["wv"] = nc.dram_tensor("wv", [E, KVL * D], BF, kind="ExternalInput")
    dr["wo"] = nc.dram_tensor("wo", [HL * D, E], BF, kind="ExternalInput")
    dr["cc"] = nc.dram_tensor("cc", [128, T], BF, kind="ExternalInput")
    dr["ss"] = nc.dram_tensor("ss", [128, T], BF, kind="ExternalInput")
    dr["perm"] = nc.dram_tensor("perm", [128, 5, 128], BF, kind="ExternalInput")
    dr["msk"] = nc.dram_tensor("msk", [128, 4, 512], BF, kind="ExternalInput")
    dr["outT"] = nc.dram_tensor("outT", [E, T], F32, kind="ExternalOutput")

    from contextlib import ExitStack
    with ExitStack() as _ctx:
        xt_sb = _ctx.enter_context(nc.sbuf_tensor("xt_sb", [128, EC, T], BF))
        wq_sb = _ctx.enter_context(nc.sbuf_tensor("wq_sb", [128, EC, HL * D], BF))
        wk_sb = _ctx.enter_context(nc.sbuf_tensor("wk_sb", [128, EC, KVL * D], BF))
        wv_sb = _ctx.enter_context(nc.sbuf_tensor("wv_sb", [128, EC, KVL * D], BF))
        wo_sb = _ctx.enter_context(nc.sbuf_tensor("wo_sb", [128, QC, E], BF))
        cc_sb = _ctx.enter_context(nc.sbuf_tensor("cc_sb", [128, T], BF))
        ss_sb = _ctx.enter_context(nc.sbuf_tensor("ss_sb", [128, T], BF))
        perm_sb = _ctx.enter_context(nc.sbuf_tensor("perm_sb", [128, 5, 128], BF))
        msk_sb = _ctx.enter_context(nc.sbuf_tensor("msk_sb", [128, 4, 512], BF))
        qraw_sb = _ctx.enter_context(nc.sbuf_tensor("qraw_sb", [128, T], BF))
        kraw_sb = _ctx.enter_context(nc.sbuf_tensor("kraw_sb", [128, T], BF))
        t1_sb = _ctx.enter_context(nc.sbuf_tensor("t1_sb", [128, 512], BF))
        t2_sb = _ctx.enter_context(nc.sbuf_tensor("t2_sb", [128, 512], BF))
        qrot_sb = _ctx.enter_context(nc.sbuf_tensor("qrot_sb", [128, QC, T], BF))
        krot_sb = _ctx.enter_context(nc.sbuf_tensor("krot_sb", [128, KVL, T], BF))
        vaug_sb = _ctx.enter_context(nc.sbuf_tensor("vaug_sb", [128, TC, KVL, 65], BF))
        pt_sb = _ctx.enter_context(nc.sbuf_tensor("pt_sb", [128, 3, 1024], BF))
        rs_sb = _ctx.enter_context(nc.sbuf_tensor("rs_sb", [128, 512], F32))
        rrep_sb = _ctx.enter_context(nc.sbuf_tensor("rrep_sb", [64, 2, 512], BF))
        rbf_sb = _ctx.enter_context(nc.sbuf_tensor("rbf_sb", [128, 512], BF))
        ones_sb = _ctx.enter_context(nc.sbuf_tensor("ones_sb", [128, 128], BF))
        ot64_sb = _ctx.enter_context(nc.sbuf_tensor("ot64_sb", [64, QC, T], BF))
        ot128_sb = _ctx.enter_context(nc.sbuf_tensor("ot128_sb", [128, QC, T], BF))
        ostg_sb = _ctx.enter_context(nc.sbuf_tensor("ostg_sb", [128, 4, 512], F32))
        ps = _ctx.enter_context(nc.psum_tensor("ps", [128, 8, 512], F32))
        dma_sem = _ctx.enter_context(nc.semaphore("dma_sem"))
        tl_sem = _ctx.enter_context(nc.semaphore("tl_sem"))
        wl_sem = _ctx.enter_context(nc.semaphore("wl_sem"))
        xl_sem = _ctx.enter_context(nc.semaphore("xl_sem"))
        pe_sem = _ctx.enter_context(nc.semaphore("pe_sem"))
        v_sem = _ctx.enter_context(nc.semaphore("v_sem"))
        s_sem = _ctx.enter_context(nc.semaphore("s_sem"))
        block = _ctx.enter_context(nc.Block())
        # "tl"/"wl"/"xl"/"dma" are all issued on the sync queue in op-list
        # order; they differ only in which semaphore the transfer bumps.
        # DMA transfers complete OUT OF ORDER (16 parallel engines), so a
        # count on one semaphore only proves "k transfers of that class
        # done". Classes are sized so consumers either wait for the whole
        # class or use a +1-transfer margin within an equal-size class.
        sems = {"pe": pe_sem, "v": v_sem, "s": s_sem, "dma": dma_sem,
                "tl": tl_sem, "wl": wl_sem, "xl": xl_sem}
        DMA_CLASSES = ("dma", "tl", "wl", "xl")
        ops = []

        def op(eng, fn, deps=(), name=None, inc=0):
            ops.append((eng, fn, inc, tuple(deps), name))

        MM = nc.tensor.matmul

        def sl(i):
            return ps[:, i, :]

        def ps2(lo):   # 2 contiguous slots as [128, 1024]
            return ps[:, lo:lo + 2, :].rearrange("p a b -> p (a b)")

        # ======== DMA loads (chunked, class semaphores) ========
        # tl class: small tables first; consumers wait the WHOLE class
        # (robust: exactly 3 transfers ever bump tl_sem).
        early_loads = [
            ("Lperm", perm_sb[:, :, :], dr["perm"][:, :, :]),
            ("Lcc", cc_sb[:, :], dr["cc"][:, :]),
            ("Lss", ss_sb[:, :], dr["ss"][:, :]),
        ]
        for nm, o, i in early_loads:
            op("tl", (lambda o=o, i=i: nc.sync.dma_start(out=o, in_=i)),
               name=nm, inc=16)
        # wl/xl classes: equal-size chunk loads; consumers of chunk e wait
        # for e+2 class completions (one-transfer reorder margin).
        xt_r = dr["xt"].rearrange("(ec p) t -> p ec t", p=128)
        wq_r = dr["wq"].rearrange("(ec p) m -> p ec m", p=128)
        for e in range(EC):
            op("wl", (lambda e=e: nc.sync.dma_start(
                out=wq_sb[:, e, :], in_=wq_r[:, e, :])),
               name=f"Lwq{e}", inc=16)
            op("xl", (lambda e=e: nc.sync.dma_start(
                out=xt_sb[:, e, :], in_=xt_r[:, e, :])),
               name=f"Lxt{e}", inc=16)
        # dma class: remaining weights; consumers wait all 4 (dep "Lwo").
        tail_loads = [
            ("Lwk", wk_sb[:, :, :], dr["wk"].rearrange("(ec p) m -> p ec m", p=128)),
            ("Lwv", wv_sb[:, :, :], dr["wv"].rearrange("(ec p) m -> p ec m", p=128)),
            ("Lmsk", msk_sb[:, :, :], dr["msk"][:, :, :]),
            ("Lwo", wo_sb[:, :, :], dr["wo"].rearrange("(qc p) e -> p qc e", p=128)),
        ]
        for nm, o, i in tail_loads:
            op("dma", (lambda o=o, i=i: nc.sync.dma_start(out=o, in_=i)),
               name=nm, inc=16)

        def ld_margin(pfx, e):
            # chunk-e dep with +1-transfer reorder margin within the class
            return f"{pfx}{min(e + 1, EC - 1)}"

        op("v", lambda: nc.vector.memset(vaug_sb[:, :, :, 64:65], 1.0),
           name="vones", inc=1)
        op("v", lambda: nc.vector.memset(ones_sb[:, :], 1.0),
           name="vones2", inc=1)
        op("v", lambda: nc.vector.memset(rbf_sb[:, :], 0.0),
           name="vzero", inc=1)

        # ======== phase A: q projections + rope ========
        # Per (c,tj): qproj bank tj; evac -> qraw; swap MM -> bank 4+tj
        # (emitted one qps-block later so the evac has time);
        # rope muls at [128,512] grain.
        def emit_qsw(c, tj):
            deps = [f"evq_{c}_{tj}", "Lss"]
            if c > 0:
                deps.append(f"qm2_{c-1}_{tj}")
            op("pe", (lambda c=c, tj=tj: MM(
                out=sl(4 + tj),
                lhsT=perm_sb[:, 0, :],
                rhs=qraw_sb[:, tj * 512:(tj + 1) * 512],
                start=True, stop=True)),
               deps=deps, name=f"qsw_{c}_{tj}", inc=1)

        for c in range(QC):
            for tj in range(TJ):
                for e in range(EC):
                    deps = [ld_margin("Lwq", e), ld_margin("Lxt", e)]
                    if e == 0 and c > 0:
                        deps.append(f"evq_{c-1}_{tj}")
                    op("pe", (lambda c=c, tj=tj, e=e: MM(
                        out=sl(tj),
                        lhsT=wq_sb[:, e, c * 128:(c + 1) * 128],
                        rhs=xt_sb[:, e, tj * 512:(tj + 1) * 512],
                        start=(e == 0), stop=(e == EC - 1))),
                       deps=deps,
                       name=(f"qps_{c}_{tj}" if e == EC - 1 else None),
                       inc=(1 if e == EC - 1 else 0))
                # swap MM of the previous (c,tj) block
                k = c * TJ + tj
                if k >= 1:
                    pc, ptj = divmod(k - 1, TJ)
                    emit_qsw(pc, ptj)
                # vector chain for this block
                edeps = [f"qps_{c}_{tj}"]
                if c > 0:
                    edeps.append(f"qsw_{c-1}_{tj}")
                op("v", (lambda tj=tj: nc.vector.tensor_copy(
                    qraw_sb[:, tj * 512:(tj + 1) * 512], sl(tj))),
                   deps=edeps, name=f"evq_{c}_{tj}", inc=1)
                op("v", (lambda tj=tj: nc.vector.tensor_mul(
                    t1_sb[:, :], qraw_sb[:, tj * 512:(tj + 1) * 512],
                    cc_sb[:, tj * 512:(tj + 1) * 512])),
                   deps=["Lss"])
                op("v", (lambda tj=tj: nc.vector.tensor_mul(
                    t2_sb[:, :], sl(4 + tj),
                    ss_sb[:, tj * 512:(tj + 1) * 512])),
                   deps=[f"qsw_{c}_{tj}", "Lss"], name=f"qm2_{c}_{tj}", inc=1)
                op("v", (lambda c=c, tj=tj: nc.vector.tensor_add(
                    qrot_sb[:, c, tj * 512:(tj + 1) * 512],
                    t1_sb[:, :], t2_sb[:, :])),
                   name=f"qrot_{c}_{tj}", inc=1)

        # ======== phase A: k projections + dup/rope ========
        for tj in range(TJ):
            for e in range(EC):
                deps = ["Lwo", f"Lxt{EC-1}"]
                if e == 0:
                    deps.append(f"evq_{QC-1}_{tj}")
                op("pe", (lambda tj=tj, e=e: MM(
                    out=sl(tj),
                    lhsT=wk_sb[:, e, :],
                    rhs=xt_sb[:, e, tj * 512:(tj + 1) * 512],
                    start=(e == 0), stop=(e == EC - 1))),
                   deps=deps,
                   name=(f"kps_{tj}" if e == EC - 1 else None),
                   inc=(1 if e == EC - 1 else 0))
            if tj == 0:
                emit_qsw(QC - 1, TJ - 1)
            op("v", (lambda tj=tj: nc.vector.tensor_copy(
                kraw_sb[:, tj * 512:(tj + 1) * 512], sl(tj))),
               deps=[f"kps_{tj}"], name=f"evk_{tj}", inc=1)

        def emit_kdup(i):
            g, tj = divmod(i, TJ)
            pp = 4 + 2 * (i % 2)
            deps = [f"evk_{tj}"]
            if i == 0:
                deps += [f"qm2_{QC-1}_{0}", f"qm2_{QC-1}_{1}"]
            elif i == 1:
                deps += [f"qm2_{QC-1}_{2}", f"qm2_{QC-1}_{3}"]
            else:
                pg, ptj = divmod(i - 2, TJ)
                deps.append(f"kro_{pg}_{ptj}")
            op("pe", (lambda g=g, tj=tj, pp=pp: MM(
                out=sl(pp),
                lhsT=perm_sb[:, 1 + g * 2, :],
                rhs=kraw_sb[:, tj * 512:(tj + 1) * 512],
                start=True, stop=True)),
               deps=deps, inc=0)
            op("pe", (lambda g=g, tj=tj, pp=pp: MM(
                out=sl(pp + 1),
                lhsT=perm_sb[:, 2 + g * 2, :],
                rhs=kraw_sb[:, tj * 512:(tj + 1) * 512],
                start=True, stop=True)),
               name=f"kdp_{g}_{tj}", inc=1)
            op("v", (lambda pp=pp, tj=tj: nc.vector.tensor_mul(
                t1_sb[:, :], sl(pp),
                cc_sb[:, tj * 512:(tj + 1) * 512])),
               deps=[f"kdp_{g}_{tj}"])
            op("v", (lambda pp=pp, tj=tj: nc.vector.tensor_mul(
                t2_sb[:, :], sl(pp + 1),
                ss_sb[:, tj * 512:(tj + 1) * 512])))
            op("v", (lambda g=g, tj=tj: nc.vector.tensor_add(
                krot_sb[:, g, tj * 512:(tj + 1) * 512],
                t1_sb[:, :], t2_sb[:, :])),
               name=f"kro_{g}_{tj}", inc=1)

        # ======== phase A: v projections (banks 0-3 rotate) ========
        def vbank(t):
            # groups 8-11 run inside the phase-B ramp on banks 4-7; the
            # rest cycle banks 0-3
            return 4 + (t - 8) if 8 <= t < 12 else t % 4

        def emit_vproj(t):
            for e in range(EC):
                deps = ["Lwo", f"Lxt{EC-1}", "vones"]
                if e == 0:
                    if t < 4:
                        deps.append(f"evk_{t}")
                    elif t < 8:
                        deps.append(f"vev_{t-4}")
                    elif t < 10:
                        deps.append("kro_1_2")
                    elif t < 12:
                        deps.append("kro_1_3")
                    else:
                        deps.append(f"vev_{t-8}")
                op("pe", (lambda t=t, e=e: MM(
                    out=ps[:, vbank(t), 0:128],
                    lhsT=xt_sb[:, e, t * 128:(t + 1) * 128],
                    rhs=wv_sb[:, e, :],
                    start=(e == 0), stop=(e == EC - 1))),
                   deps=deps,
                   name=(f"vps_{t}" if e == EC - 1 else None),
                   inc=(1 if e == EC - 1 else 0))
            for g in range(KVL):
                op("v", (lambda t=t, g=g: nc.vector.tensor_copy(
                    vaug_sb[:, t, g, 0:64],
                    ps[:, vbank(t), g * 64:(g + 1) * 64])),
                   deps=([f"vps_{t}"] if g == 0 else ()),
                   name=(f"vev_{t}" if g == KVL - 1 else None),
                   inc=(1 if g == KVL - 1 else 0))

        # interleave kdups with vproj groups so the PE never waits on the
        # kro DVE chain
        emit_kdup(0)
        emit_kdup(1)
        for t in range(6):
            emit_vproj(t)
            if t + 2 < 8:
                emit_kdup(t + 2)
        for t in (6, 7, 12, 13, 14, 15):
            emit_vproj(t)

        # ======== phase B: attention, flat chunk-pair pipeline ========
        pairs = []
        for lh in range(HL):
            for qj in range(TJ):
                npair = 2 * qj + 2
                for kp in range(npair):
                    pairs.append((lh, qj, kp, npair))
        P = len(pairs)

        def emit_st(p):
            lh, qj, kp, npair = pairs[p]
            g, pq, c = lh // 4, lh % 2, lh // 2
            sbase = 2 * (p % 2)
            for j in range(2):
                ki = kp * 2 + j
                deps = []
                if j == 0:
                    deps += [f"qrot_{c}_{qj}",
                             f"kro_{g}_{(2*kp)//4}", f"kro_{g}_{(2*kp+1)//4}"]
                    if p >= 2:
                        deps.append(f"exp_{p-2}")
                    else:
                        deps += [f"vev_{12 + 2*p}", f"vev_{13 + 2*p}"]
                op("pe", (lambda sbase=sbase, j=j, ki=ki, pq=pq, g=g, c=c, qj=qj: MM(
                    out=sl(sbase + j),
                    lhsT=krot_sb[pq * 64:(pq + 1) * 64, g,
                                 ki * 128:(ki + 1) * 128],
                    rhs=qrot_sb[pq * 64:(pq + 1) * 64, c,
                                qj * 512:(qj + 1) * 512],
                    start=True, stop=True)),
                   deps=deps,
                   name=(f"st_{p}" if j == 1 else None),
                   inc=(1 if j == 1 else 0))

        def emit_av(p):
            lh, qj, kp, npair = pairs[p]
            g, c, idx = lh // 4, lh // 2, lh * TJ + qj
            nk = 4 * qj + 4
            oslot = 4 + idx % 2
            avdep = f"msk_{p}" if kp >= 2 * qj else f"exp_{p}"
            for j in range(2):
                ki = kp * 2 + j
                deps = []
                if j == 0:
                    deps = [avdep, f"vev_{2*kp}", f"vev_{2*kp+1}"]
                    if kp == 0 and idx >= 2:
                        deps.append(f"onorm_{idx-2}")
                    elif idx == 0 and kp == 0:
                        deps.append("vev_8")
                    elif idx == 1 and kp == 0:
                        deps.append("vev_9")
                op("pe", (lambda oslot=oslot, p=p, j=j, ki=ki, g=g, nk=nk: MM(
                    out=ps[0:65, oslot, :],
                    lhsT=vaug_sb[:, ki, g, :],
                    rhs=pt_sb[:, p % 3, j * 512:(j + 1) * 512],
                    start=(ki == 0), stop=(ki == nk - 1))),
                   deps=deps,
                   name=(f"av_{p}" if j == 1 else None),
                   inc=(1 if j == 1 else 0))

        def emit_exp_msk(p):
            lh, qj, kp, npair = pairs[p]
            sbase = 2 * (p % 2)
            deps = [f"st_{p}"]
            if p >= 3:
                deps.append(f"av_{p-3}")
            op("s", (lambda sbase=sbase, p=p: nc.scalar.activation(
                out=pt_sb[:, p % 3, :],
                in_=ps2(sbase),
                func=mybir.ActivationFunctionType.Exp,
                scale=1.0)),
               deps=deps, name=f"exp_{p}", inc=1)
            if kp >= 2 * qj:   # diagonal pair: causal mask
                for j in range(2):
                    r = kp * 2 + j - 4 * qj
                    op("v", (lambda p=p, j=j, r=r: nc.vector.tensor_mul(
                        pt_sb[:, p % 3, j * 512:(j + 1) * 512],
                        pt_sb[:, p % 3, j * 512:(j + 1) * 512],
                        msk_sb[:, r, :])),
                       deps=([f"exp_{p}", "Lwo"] if j == 0 else ()),
                       name=(f"msk_{p}" if j == 1 else None),
                       inc=(1 if j == 1 else 0))

        def emit_row_end(p):
            # after av_p of the last pair of (lh,qj): recip + copy on DVE
            lh, qj, kp, npair = pairs[p]
            idx = lh * TJ + qj
            oslot = 4 + idx % 2
            op("v", (lambda oslot=oslot: nc.vector.reciprocal(
                out=rs_sb[64:65, :], in_=ps[64:65, oslot, :])),
               deps=[f"av_{p}"])
            op("v", (lambda: nc.vector.tensor_copy(
                rbf_sb[64:65, :], rs_sb[64:65, :])),
               deps=([f"rbc_{idx-1}"] if idx >= 1 else ()),
               name=f"rcp_{idx}", inc=1)

        def emit_rbc(idx):
            # PE broadcast of 1/denom, bank 6+idx%2.  Same (64,128) tile
            # mode as the ST matmuls: contraction over partitions 64:128
            # of rbf where row 64 holds 1/denom and rows 65:128 are zero.
            bslot = 6 + idx % 2
            deps = [f"rcp_{idx}", "vones2"]
            if idx >= 2:
                deps.append(f"onorm_{idx-2}")
            elif idx == 0:
                deps.append("vev_10")
            else:
                deps.append("vev_11")
            op("pe", (lambda bslot=bslot: MM(
                out=ps[:, bslot, :],
                lhsT=ones_sb[64:128, :],
                rhs=rbf_sb[64:128, :],
                start=True, stop=True)),
               deps=deps, name=f"rbc_{idx}", inc=1)
            lh, qj = divmod(idx, TJ)
            oslot = 4 + idx % 2
            op("v", (lambda idx=idx, bslot=bslot: nc.vector.tensor_copy(
                rrep_sb[:, idx % 2, :], ps[0:64, bslot, :])),
               deps=[f"rbc_{idx}"])
            if lh % 2 == 0:
                tgt = ot128_sb[0:64, lh // 2, qj * 512:(qj + 1) * 512]
            else:
                tgt = ot64_sb[:, lh // 2, qj * 512:(qj + 1) * 512]
            op("v", (lambda tgt=tgt, oslot=oslot, idx=idx: nc.vector.tensor_mul(
                tgt,
                ps[0:64, oslot, :],
                rrep_sb[:, idx % 2, :])),
               name=f"onorm_{idx}", inc=1)

        # pipeline: ST two pairs ahead of AV; rbc deferred ~2 pairs and
        # placed inside the 64-tile block (rbc+ST) to avoid extra mode
        # switches; V-proj groups 8-11 fill the ramp so the PE stays busy
        # (and HAM-warm) while the first exps run.
        emit_st(0)
        emit_exp_msk(0)
        emit_st(1)
        emit_exp_msk(1)
        pend_rbc = []
        for p in range(P):
            if p < 4:
                emit_vproj(8 + p)
            emit_av(p)
            if pend_rbc and pend_rbc[0][1] <= p:
                emit_rbc(pend_rbc.pop(0)[0])
            if p + 2 < P:
                emit_st(p + 2)
                emit_exp_msk(p + 2)
            lh, qj, kp, npair = pairs[p]
            if kp == npair - 1:
                emit_row_end(p)
                pend_rbc.append((lh * TJ + qj, p + 2))
        for idx, _ in pend_rbc:
            emit_rbc(idx)

        # ======== lift: odd-head OT 0:64 -> ot128 64:128 via SBUF DMA ========
        for cq in range(QC):
            for tj in range(TJ):
                idx = (2 * cq + 1) * TJ + tj
                op("dma", (lambda cq=cq, tj=tj: nc.sync.dma_start(
                    out=ot128_sb[64:128, cq, tj * 512:(tj + 1) * 512],
                    in_=ot64_sb[0:64, cq, tj * 512:(tj + 1) * 512])),
                   deps=[f"onorm_{idx}"], name=f"dml_{cq}_{tj}", inc=16)

        # ======== phase C: output projection ========
        for tj in range(TJ):
            for m in range(EC):
                idc = tj * EC + m
                bank = idc % 4
                deps0 = [f"onorm_{6*TJ+tj}", f"dml_{QC-1}_{TJ-1}"]
                if idc >= 4:
                    deps0.append(f"oev_{idc-4}")
                else:
                    deps0.append(f"exp_{P-1}")
                for kc in range(QC):
                    op("pe", (lambda m=m, tj=tj, kc=kc, bank=bank: MM(
                        out=sl(bank),
                        lhsT=wo_sb[:, kc, m * 128:(m + 1) * 128],
                        rhs=ot128_sb[:, kc, tj * 512:(tj + 1) * 512],
                        start=(kc == 0), stop=(kc == QC - 1))),
                       deps=(deps0 if kc == 0 else ()),
                       name=(f"opj_{idc}" if kc == QC - 1 else None),
                       inc=(1 if kc == QC - 1 else 0))
                edeps = [f"opj_{idc}"]
                if idc >= 4:
                    edeps.append(f"ost_{idc-4}")
                op("v", (lambda idc=idc, bank=bank: nc.vector.tensor_copy(
                    ostg_sb[:, idc % 4, :], sl(bank))),
                   deps=edeps, name=f"oev_{idc}", inc=1)
                op("dma", (lambda m=m, tj=tj, idc=idc: nc.sync.dma_start(
                    out=dr["outT"][m * 128:(m + 1) * 128,
                                   tj * 512:(tj + 1) * 512],
                    in_=ostg_sb[:, idc % 4, :])),
                   deps=[f"oev_{idc}"], name=f"ost_{idc}", inc=16)

        # ---------- resolve ticks ----------
        tick = {"pe": 0, "v": 0, "s": 0, "dma": 0,
                "tl": 0, "wl": 0, "xl": 0}
        ev = {}
        for eng, fn, inc, deps, name in ops:
            tick[eng] += inc
            if name:
                ev[name] = (eng, tick[eng])
        total = dict(tick)

        def emit(block_eng, eng_keys):
            waited = {}
            for eng, fn, inc, deps, name in ops:
                if eng not in eng_keys:
                    continue
                for dname in deps:
                    deng, dtick = ev[dname]
                    if deng in eng_keys or waited.get(deng, -1) >= dtick:
                        continue
                    block_eng.wait_ge(sems[deng], dtick)
                    waited[deng] = dtick
                inst = fn()
                if inc:
                    inst.then_inc(sems[eng], inc)

        @block.sync
        def _(sync):
            emit(sync, set(DMA_CLASSES))
            for k in DMA_CLASSES:
                if total[k]:
                    sync.wait_ge(sems[k], total[k])

        emit_one = lambda key: (lambda blk: emit(blk, {key}))

        @block.tensor
        def _(tensor):
            emit(tensor, {"pe"})

        @block.vector
        def _(vector):
            emit(vector, {"v"})

        @block.scalar
        def _(scalar):
            emit(scalar, {"s"})

    return nc


def _host_tables():
    inv = 1.0 / (10000.0 ** (np.arange(0, D, 2, dtype=np.float64) / D))
    ang = np.arange(T, dtype=np.float64)[:, None] * inv[None, :]
    cos = np.cos(ang).T
    sin = np.sin(ang).T
    c64 = np.repeat(cos, 2, axis=0)
    s64 = np.empty((64, T))
    s64[0::2] = -sin
    s64[1::2] = sin
    cc = np.tile(c64, (2, 1)).astype(np.float32)
    ss = np.tile(s64, (2, 1)).astype(np.float32)

    def swap(r):
        return r + 1 if r % 2 == 0 else r - 1

    perm = np.zeros((128, 5, 128), np.float32)
    for r in range(128):
        perm[swap(r), 0, r] = 1.0
    for gg in range(2):
        for sw in range(2):
            for r in range(128):
                d_ = r % 64
                k = gg * 64 + (swap(d_) if sw else d_)
                perm[k, 1 + gg * 2 + sw, r] = 1.0

    msk = np.zeros((4, 128, 512), np.float32)
    for r in range(4):
        for p in range(128):
            v = r * 128 + p
            if v < 512:
                msk[r, p, v:] = 1.0
    msk = np.ascontiguousarray(msk.transpose(1, 0, 2))
    return cc, ss, perm, msk


def kernel(x, freq_cis, Wq, Wk, Wv, Wo):
    x = np.asarray(x, np.float32)
    Wq = np.asarray(Wq, np.float32)
    Wk = np.asarray(Wk, np.float32)
    Wv = np.asarray(Wv, np.float32)
    Wo = np.asarray(Wo, np.float32)
    bf16 = ml_dtypes.bfloat16

    if "nc" not in _CACHE:
        _CACHE["nc"] = _build()
    nc = _CACHE["nc"]

    cc, ss, perm, msk = _host_tables()
    in_maps = []
    for core in range(8):
        b, hg = core // 4, core % 4
        in_maps.append({
            "xt": np.ascontiguousarray(x[b].T).astype(bf16),
            "wq": (Wq[:, hg * 512:(hg + 1) * 512] / 8.0).astype(bf16),
            "wk": Wk[:, hg * 128:(hg + 1) * 128].astype(bf16),
            "wv": Wv[:, hg * 128:(hg + 1) * 128].astype(bf16),
            "wo": Wo[hg * 512:(hg + 1) * 512, :].astype(bf16),
            "cc": cc.astype(bf16), "ss": ss.astype(bf16),
            "perm": perm.astype(bf16), "msk": msk.astype(bf16),
        })

    _CACHE["in_maps"] = in_maps
    res = run_bass_kernel_spmd(nc, in_maps, list(range(8)))
    out = np.zeros((B, T, E), np.float32)
    for core in range(8):
        b = core // 4
        out[b] += res.results[core]["outT"].T
    return out
